# revision 1
# baseline (speedup 1.0000x reference)
"""Trainium2 Bass kernel for nn_CosineLoss (cosine-similarity pseudo-label CE loss).

Data-parallel over the flattened (B*P) patch dimension across 8 NeuronCores.

Per core the device computes, for each patch x (row of features):
  q_c  = dot(x, a_c / ||a_c||)   for the 4 prototypes   (PE, fp32r matmuls)
  n2   = ||x||^2                 (squares on ACT/DVE -> bf16, reduced on PE
                                  against a ones vector)
  keep = (q_0 > q_l) & (q_0 > 0) & (q_0^2 > 0.36 * n2)    [== sim_back>sim_sea
                                                           & sim_back>0.6]
  pseudo = is_foreground & ~keep
  s    = softmax(z); lse2 = log(sum(exp(s)))            (double-softmax CE)
  pp   = pseudo ? w_l*(lse2-s_l) : w_0*(lse2-s_0)       (masked for padding)
and returns per-partition partial sums of pp; the host adds them up and
divides by B*P.

Features are supplied to the device pre-packed so the contraction dim D lands
on SBUF partitions and each per-group DMA is one long contiguous run per
partition; everything else is index prep on tiny tensors.
"""

import numpy as np
from contextlib import ExitStack

import concourse.bass as bass
import concourse.bacc as bacc
import concourse.tile as tile
from concourse import mybir
from concourse.bass_utils import run_bass_kernel_spmd

# Problem constants (hardcoded; kernel.py must be self-contained).
B, P, D, C = 512, 45, 2048, 4
EPS = 1e-8
THRESH2 = 0.36  # THRESH**2, THRESH = 0.6
NCORES = 8
ROWS = B * P                 # 23040 patches
RT = 23                      # row tiles of 128 per core
R = RT * 128                 # 2944 padded rows per core
K = D // 128                 # 16 contraction chunks
GROUPS = [(0, 512), (512, 512), (1024, 512), (1536, 512), (2048, 512), (2560, 384)]
SQ_SPLIT = 3                 # of 4 square-ops per group: first 3 on ACT, last on DVE

F32 = mybir.dt.float32
F32R = mybir.dt.float32r
BF16 = mybir.dt.bfloat16
AF = mybir.ActivationFunctionType
ALU = mybir.AluOpType
AXX = mybir.AxisListType.X

_CACHE = {}


def _build():
    nc = bacc.Bacc("TRN2", target_bir_lowering=False, debug=False)
    gsz = K * R
    featg = nc.dram_tensor("featg", [128, gsz], F32R, kind="ExternalInput").ap()
    avgtn = nc.dram_tensor("avgtn", [128, K * C], F32R, kind="ExternalInput").ap()
    zrow = nc.dram_tensor("zrow", [128, RT * C], F32, kind="ExternalInput").ap()
    meta = nc.dram_tensor("meta", [128, RT * 8], F32, kind="ExternalInput").ap()
    eye5 = nc.dram_tensor("eye5", [5, 5], F32, kind="ExternalInput").ap()
    out = nc.dram_tensor("out", [128, 1], F32, kind="ExternalOutput").ap()

    with tile.TileContext(nc) as tc, ExitStack() as ctx:
        consts = ctx.enter_context(tc.tile_pool(name="consts", bufs=1))
        gpool = ctx.enter_context(tc.tile_pool(name="gpool", bufs=2))
        sqpool = ctx.enter_context(tc.tile_pool(name="sqpool", bufs=2))
        sb = ctx.enter_context(tc.tile_pool(name="sb", bufs=1))
        qps = ctx.enter_context(tc.tile_pool(name="qps", bufs=2, space="PSUM"))
        nps = ctx.enter_context(tc.tile_pool(name="nps", bufs=2, space="PSUM"))
        tps = ctx.enter_context(tc.tile_pool(name="tps", bufs=1, space="PSUM"))

        _tcnt = [0]

        def t23(pool=sb, shape=(128, RT), dt=F32):
            _tcnt[0] += 1
            nm = f"tmp_{_tcnt[0]}"
            return pool.tile(list(shape), dt, name=nm, tag=nm)

        # ---- constants / small inputs ----
        avgtn_sb = consts.tile([128, K, C], F32R)
        nc.sync.dma_start(out=avgtn_sb, in_=avgtn.rearrange("p (k c) -> p k c", c=C))
        eye5_sb = consts.tile([5, 5], F32)
        nc.sync.dma_start(out=eye5_sb, in_=eye5)
        eye4 = eye5_sb[0:4, 0:4]
        eye1 = eye5_sb[0:1, 0:1]
        ones_sb = consts.tile([128, 1], BF16)
        nc.vector.memset(ones_sb, 1.0)
        zsb = sb.tile([128, RT, C], F32)
        nc.sync.dma_start(out=zsb, in_=zrow.rearrange("p (t c) -> p t c", c=C))
        msb = sb.tile([128, RT, 8], F32)
        nc.sync.dma_start(out=msb, in_=meta.rearrange("p (t c) -> p t c", c=8))

        oh = msb[:, :, 0:4]
        wl = msb[:, :, 4]
        fgv = msb[:, :, 5]
        w0v = msb[:, :, 6]

        # ---- z-only epilogue half, hoisted to the front (overlaps group DMAs,
        # and pulls the ACT exp/ln table loads off the tail) ----
        e = sb.tile([128, RT, C], F32)
        nc.scalar.activation(e, zsb, AF.Exp)
        zsum = t23()
        nc.vector.reduce_sum(zsum, e, axis=AXX)
        rz = t23()
        nc.vector.reciprocal(rz, zsum)
        s = sb.tile([128, RT, C], F32)
        nc.vector.tensor_mul(s, e, rz.unsqueeze(2).broadcast_to([128, RT, C]))
        es = sb.tile([128, RT, C], F32)
        nc.scalar.activation(es, s, AF.Exp)
        essum = t23()
        nc.vector.reduce_sum(essum, es, axis=AXX)
        lse2 = t23()
        nc.scalar.activation(lse2, essum, AF.Ln)
        soh = sb.tile([128, RT, C], F32)
        nc.vector.tensor_mul(soh, s, oh)
        sl = t23()
        nc.vector.reduce_sum(sl, soh, axis=AXX)
        base = t23()
        nc.vector.tensor_sub(base, lse2, s[:, :, 0])
        alt = t23()
        nc.vector.tensor_sub(alt, lse2, sl)
        b1 = t23()
        nc.vector.tensor_mul(b1, w0v, base)
        a1 = t23()
        nc.vector.tensor_mul(a1, wl, alt)
        dd = t23()
        nc.vector.tensor_sub(dd, a1, b1)

        # ---- main feature stream: per patch-group DMA -> squares -> matmuls ->
        # stage -> transpose -> qn slices ----
        qn = sb.tile([128, RT, 4], F32)
        n2t = sb.tile([128, RT], F32)
        goff = 0
        for gi, (off, w) in enumerate(GROUPS):
            g = gpool.tile([128, K, w], F32R, name=f"g{gi}", tag="g")
            nc.sync.dma_start(
                out=g,
                in_=featg[:, goff:goff + K * w].rearrange("p (k r) -> p k r", r=w))
            goff += K * w
            sq = sqpool.tile([128, K, w], BF16, name=f"sq{gi}", tag="sq")
            for j in range(4):
                src = g[:, j * 4:(j + 1) * 4, :].bitcast(F32)
                dst = sq[:, j * 4:(j + 1) * 4, :]
                if j < SQ_SPLIT:
                    nc.scalar.activation(dst, src, AF.Square)
                else:
                    nc.vector.tensor_mul(dst, src, src)
            pq = qps.tile([C, w], F32, name=f"pq{gi}", tag="pq")
            for k in range(K):
                nc.tensor.matmul(pq, avgtn_sb[:, k, :], g[:, k, :],
                                 start=(k == 0), stop=(k == K - 1))
            pn = nps.tile([1, w], F32, name=f"pn{gi}", tag="pn")
            for k in range(K):
                nc.tensor.matmul(pn, ones_sb, sq[:, k, :],
                                 start=(k == 0), stop=(k == K - 1))
            stq = t23(shape=(4, w))
            nc.vector.tensor_copy(stq, pq)
            stn = t23(shape=(1, w))
            nc.vector.tensor_copy(stn, pn)
            # per-group small transposes: [4, w] -> w/128 tiles of [128, 4]
            nt = w // 128
            t0 = off // 128
            ptq = tps.tile([128, nt * 4], F32, name=f"ptq{gi}", tag="ptq")
            ptn = tps.tile([128, nt], F32, name=f"ptn{gi}", tag="ptn")
            for j in range(nt):
                nc.tensor.transpose(ptq[:, j * 4:(j + 1) * 4],
                                    stq[:, j * 128:(j + 1) * 128], eye4)
                nc.tensor.transpose(ptn[:, j:j + 1],
                                    stn[:, j * 128:(j + 1) * 128], eye1)
            nc.vector.tensor_copy(
                qn[:, t0:t0 + nt, :].rearrange("p t c -> p (t c)"), ptq)
            nc.vector.tensor_copy(n2t[:, t0:t0 + nt], ptn)

        # ---- q-dependent epilogue (tail) ----
        q0 = qn[:, :, 0]
        ql = t23()
        qoh = sb.tile([128, RT, C], F32)
        nc.vector.tensor_mul(qoh, qn, oh)
        nc.vector.reduce_sum(ql, qoh, axis=AXX)
        c1 = t23()
        nc.vector.tensor_tensor(c1, q0, ql, op=ALU.is_gt)
        q0sq = t23()
        nc.vector.tensor_mul(q0sq, q0, q0)
        t2 = t23()
        nc.vector.tensor_scalar_mul(t2, n2t, THRESH2)
        c2a = t23()
        nc.vector.tensor_scalar(c2a, q0, 0.0, None, op0=ALU.is_gt)
        c2b = t23()
        nc.vector.tensor_tensor(c2b, q0sq, t2, op=ALU.is_gt)
        keep = t23()
        nc.vector.tensor_mul(keep, c1, c2a)
        keep2 = t23()
        nc.vector.tensor_mul(keep2, keep, c2b)
        fk = t23()
        nc.vector.tensor_mul(fk, fgv, keep2)
        pv = t23()
        nc.vector.tensor_sub(pv, fgv, fk)
        t3 = t23()
        nc.vector.tensor_mul(t3, pv, dd)
        pp = t23()
        nc.vector.tensor_add(pp, t3, b1)
        rowsum = sb.tile([128, 1], F32)
        nc.vector.reduce_sum(rowsum, pp, axis=AXX)
        nc.sync.dma_start(out=out, in_=rowsum)

    nc.compile()
    return nc


def _prep(features, average_features, outputs, labels_onehot, weights):
    feats = np.asarray(features, np.float32).reshape(ROWS, D)
    z = np.asarray(outputs, np.float32).reshape(ROWS, C)
    lab = np.asarray(labels_onehot, np.float32)
    w = np.asarray(weights, np.float32)
    avg = np.asarray(average_features, np.float32)

    l_img = np.argmax(lab, axis=1)
    lp = np.repeat(l_img, P)                                    # [23040]
    an = avg / np.maximum(np.linalg.norm(avg, axis=1, keepdims=True), EPS)

    npad = NCORES * R
    zp = np.zeros((npad, C), np.float32)
    zp[:ROWS] = z
    meta = np.zeros((npad, 8), np.float32)
    meta[:ROWS, 0:4] = np.eye(C, dtype=np.float32)[lp]
    meta[:ROWS, 4] = w[lp]
    meta[:ROWS, 5] = (lp > 0).astype(np.float32)
    meta[:ROWS, 6] = w[0]

    avgtn = np.ascontiguousarray(
        an.T.reshape(K, 128, C).transpose(1, 0, 2).reshape(128, K * C))
    eye5 = np.eye(5, dtype=np.float32)

    # Packed feature layout: per core, per group (off, w):
    # featg[p, goff + k*w + r] = feats[core*R + off + r, k*128 + p]
    featsP = np.zeros((npad, D), np.float32)
    featsP[:ROWS] = feats
    u = featsP.reshape(NCORES, R, K, 128)                       # [core, r, k, p]
    parts = []
    for off, w in GROUPS:
        blk = u[:, off:off + w]                                 # [core, w, k, p]
        parts.append(blk.transpose(0, 3, 2, 1).reshape(NCORES, 128, K * w))
    featg_all = np.ascontiguousarray(np.concatenate(parts, axis=2))

    in_maps = []
    for ci in range(NCORES):
        lo, hi = ci * R, (ci + 1) * R
        zrow = np.ascontiguousarray(
            zp[lo:hi].reshape(RT, 128, C).transpose(1, 0, 2).reshape(128, RT * C))
        metar = np.ascontiguousarray(
            meta[lo:hi].reshape(RT, 128, 8).transpose(1, 0, 2).reshape(128, RT * 8))
        in_maps.append({"featg": featg_all[ci], "avgtn": avgtn,
                        "zrow": zrow, "meta": metar, "eye5": eye5})
    return in_maps


def kernel(features, average_features, outputs, labels_onehot, weights,
           _trace=False, _trace_kwargs=None):
    if "nc" not in _CACHE:
        _CACHE["nc"] = _build()
    nc = _CACHE["nc"]
    in_maps = _prep(features, average_features, outputs, labels_onehot, weights)
    kwargs = {}
    if _trace:
        kwargs = dict(trace=True, **(_trace_kwargs or {}))
    res = run_bass_kernel_spmd(nc, in_maps, core_ids=list(range(NCORES)), **kwargs)
    total = np.float64(0.0)
    for r in res.results:
        total += np.float64(r["out"].sum())
    _CACHE["last_results"] = res
    return np.float32(total / ROWS)



# revision 5
# speedup vs baseline: 176990.6275x; 176990.6275x over previous
"""Trainium2 Bass kernel for nn_CosineLoss (cosine-similarity pseudo-label CE loss).

Data-parallel over the flattened (B*P) patch dimension across 8 NeuronCores.

Per core the device computes, for each patch x (row of features):
  q_c  = dot(x, a_c / ||a_c||)   for the 4 prototypes   (PE, bf16 matmuls)
  n2   = ||x||^2                 (bf16 squares on ACT/DVE, reduced on PE
                                  against a ones vector)
  keep = (q_0 > q_l) & (q_0 > 0) & (q_0^2 > 0.36 * n2)    [== sim_back>sim_sea
                                                           & sim_back>0.6]
  pseudo = is_foreground & ~keep
  s    = softmax(z); lse2 = log(sum(exp(s)))            (double-softmax CE)
  pp   = pseudo ? w_l*(lse2-s_l) : w_0*(lse2-s_0)       (masked for padding)
and returns per-partition partial sums of pp; the host adds them up and
divides by B*P.

Features are converted to bf16 on the host (the loss only consumes them
through three comparisons with huge margins, so bf16 is far inside the
accuracy budget) and pre-packed so the contraction dim D lands on SBUF
partitions; this halves HBM traffic and doubles PE streaming rate vs fp32.
"""

import numpy as np
from contextlib import ExitStack

import concourse.bass as bass
import concourse.bacc as bacc
import concourse.tile as tile
from concourse import mybir
from concourse.bass_utils import run_bass_kernel_spmd

# Problem constants (hardcoded; kernel.py must be self-contained).
B, P, D, C = 512, 45, 2048, 4
THRESH2 = 0.36  # THRESH**2, THRESH = 0.6
NCORES = 8
ROWS = B * P                 # 23040 patches
RT = 23                      # row tiles of 128 per core
R = RT * 128                 # 2944 padded rows per core
K = D // 128                 # 16 contraction chunks
GROUPS = [(0, 512), (512, 512), (1024, 512), (1536, 512), (2048, 512), (2560, 384)]
NSUB = 4                     # sub-DMAs per group (4 chunks each)
SQ_ACT_QUADS = 2             # of 4 square-quads per group: first 2 on ACT, rest DVE

F32 = mybir.dt.float32
BF16 = mybir.dt.bfloat16
AF = mybir.ActivationFunctionType
ALU = mybir.AluOpType
AXX = mybir.AxisListType.X

_CACHE = {}


def _build():
    nc = bacc.Bacc("TRN2", target_bir_lowering=False, debug=False)
    gsz = K * R
    featg = nc.dram_tensor("featg", [128, gsz], BF16, kind="ExternalInput").ap()
    avgtn = nc.dram_tensor("avgtn", [128, K * C], BF16, kind="ExternalInput").ap()
    zrow = nc.dram_tensor("zrow", [128, RT * C], F32, kind="ExternalInput").ap()
    meta = nc.dram_tensor("meta", [128, RT * 8], F32, kind="ExternalInput").ap()
    eye5 = nc.dram_tensor("eye5", [5, 5], F32, kind="ExternalInput").ap()
    out = nc.dram_tensor("out", [128, 1], F32, kind="ExternalOutput").ap()

    with tile.TileContext(nc) as tc, ExitStack() as ctx:
        consts = ctx.enter_context(tc.tile_pool(name="consts", bufs=1))
        gpool = ctx.enter_context(tc.tile_pool(name="gpool", bufs=3))
        sqpool = ctx.enter_context(tc.tile_pool(name="sqpool", bufs=3))
        sb = ctx.enter_context(tc.tile_pool(name="sb", bufs=1))
        qps = ctx.enter_context(tc.tile_pool(name="qps", bufs=2, space="PSUM"))
        tps = ctx.enter_context(tc.tile_pool(name="tps", bufs=2, space="PSUM"))

        _tcnt = [0]

        def t23(pool=sb, shape=(128, RT), dt=F32):
            _tcnt[0] += 1
            nm = f"tmp_{_tcnt[0]}"
            return pool.tile(list(shape), dt, name=nm, tag=nm)

        # ---- small constants first (they gate every matmul / the z path) ----
        avgtn_sb = consts.tile([128, K, C], BF16)
        nc.sync.dma_start(out=avgtn_sb, in_=avgtn.rearrange("p (k c) -> p k c", c=C))
        eye5_sb = consts.tile([5, 5], F32)
        nc.sync.dma_start(out=eye5_sb, in_=eye5)
        # z15: [0,0,0,0,1] columns — the n2 pass streams sq through this so
        # row 4 of the shared psum tile picks up sum(x^2) while rows 0-3 get
        # an additive zero (they later accumulate the q matmuls).
        z15 = consts.tile([128, 5], BF16)
        nc.vector.memset(z15[:, 0:4], 0.0)
        nc.vector.memset(z15[:, 4:5], 1.0)

        # ---- feature stream DMAs, issued ahead of everything bulky ----
        gtiles = []
        goff = 0
        for gi, (off, w) in enumerate(GROUPS):
            g = gpool.tile([128, K, w], BF16, name=f"g{gi}", tag="g")
            gtiles.append(g)
            sub = K // NSUB
            for s in range(NSUB):
                nc.sync.dma_start(
                    out=g[:, s * sub:(s + 1) * sub, :],
                    in_=featg[:, goff + s * sub * w: goff + (s + 1) * sub * w]
                        .rearrange("p (k r) -> p k r", r=w))
            goff += K * w

        zsb = sb.tile([128, RT, C], F32)
        nc.sync.dma_start(out=zsb, in_=zrow.rearrange("p (t c) -> p t c", c=C))
        msb = sb.tile([128, RT, 8], F32)
        nc.sync.dma_start(out=msb, in_=meta.rearrange("p (t c) -> p t c", c=8))

        oh = msb[:, :, 0:4]
        wl = msb[:, :, 4]
        fgv = msb[:, :, 5]
        w0v = msb[:, :, 6]

        # ---- z-only epilogue half, hoisted to the front (overlaps group DMAs,
        # and pulls the ACT exp/ln table loads off the tail) ----
        e = sb.tile([128, RT, C], F32)
        nc.scalar.activation(e, zsb, AF.Exp)
        zsum = t23()
        nc.vector.reduce_sum(zsum, e, axis=AXX)
        rz = t23()
        nc.vector.reciprocal(rz, zsum)
        s = sb.tile([128, RT, C], F32)
        nc.vector.tensor_mul(s, e, rz.unsqueeze(2).broadcast_to([128, RT, C]))
        es = sb.tile([128, RT, C], F32)
        nc.scalar.activation(es, s, AF.Exp)
        essum = t23()
        nc.vector.reduce_sum(essum, es, axis=AXX)
        lse2 = t23()
        nc.scalar.activation(lse2, essum, AF.Ln)
        soh = sb.tile([128, RT, C], F32)
        nc.vector.tensor_mul(soh, s, oh)
        sl = t23()
        nc.vector.reduce_sum(sl, soh, axis=AXX)
        base = t23()
        nc.vector.tensor_sub(base, lse2, s[:, :, 0])
        alt = t23()
        nc.vector.tensor_sub(alt, lse2, sl)
        b1 = t23()
        nc.vector.tensor_mul(b1, w0v, base)
        a1 = t23()
        nc.vector.tensor_mul(a1, wl, alt)
        dd = t23()
        nc.vector.tensor_sub(dd, a1, b1)

        # ---- main feature stream: squares -> matmuls -> stage -> transpose ->
        # per-group epilogue ----
        pps = sb.tile([128, RT], F32)
        for gi, (off, w) in enumerate(GROUPS):
            g = gtiles[gi]
            sq = sqpool.tile([128, K, w], BF16, name=f"sq{gi}", tag="sq")
            for j in range(4):
                src = g[:, j * 4:(j + 1) * 4, :]
                dst = sq[:, j * 4:(j + 1) * 4, :]
                if j < SQ_ACT_QUADS:
                    nc.scalar.activation(dst, src, AF.Square)
                else:
                    nc.vector.tensor_mul(dst, src, src)
            pq = qps.tile([5, w], F32, name=f"pq{gi}", tag="pq")
            for k in range(K):
                nc.tensor.matmul(pq, z15, sq[:, k, :],
                                 start=(k == 0), stop=False)
            for k in range(K):
                nc.tensor.matmul(pq[0:4, :], avgtn_sb[:, k, :], g[:, k, :],
                                 start=False, stop=(k == K - 1))
            stq = t23(shape=(5, w))
            nc.vector.tensor_copy(stq, pq)
            # per-group small transposes: [5, w] -> w/128 tiles of [128, 5]
            nt = w // 128
            t0 = off // 128
            ptq = tps.tile([128, nt * 5], F32, name=f"ptq{gi}", tag="ptq")
            for j in range(nt):
                nc.tensor.transpose(ptq[:, j * 5:(j + 1) * 5],
                                    stq[:, j * 128:(j + 1) * 128], eye5_sb)
            qn = t23(shape=(128, nt, 5))
            nc.vector.tensor_copy(qn, ptq.rearrange("p (t c) -> p t c", c=5))

            # ---- q-dependent epilogue for this group's row tiles ----
            ohg = oh[:, t0:t0 + nt, :]
            q0 = qn[:, :, 0]
            qoh = t23(shape=(128, nt, 4))
            nc.vector.tensor_mul(qoh, qn[:, :, 0:4], ohg)
            ql = t23(shape=(128, nt))
            nc.vector.reduce_sum(ql, qoh, axis=AXX)
            c1 = t23(shape=(128, nt))
            nc.vector.tensor_tensor(c1, q0, ql, op=ALU.is_gt)
            q0sq = t23(shape=(128, nt))
            nc.vector.tensor_mul(q0sq, q0, q0)
            t2 = t23(shape=(128, nt))
            nc.vector.tensor_scalar_mul(t2, qn[:, :, 4], THRESH2)
            c2a = t23(shape=(128, nt))
            nc.vector.tensor_scalar(c2a, q0, 0.0, None, op0=ALU.is_gt)
            c2b = t23(shape=(128, nt))
            nc.vector.tensor_tensor(c2b, q0sq, t2, op=ALU.is_gt)
            keep = t23(shape=(128, nt))
            nc.vector.tensor_mul(keep, c1, c2a)
            keep2 = t23(shape=(128, nt))
            nc.vector.tensor_mul(keep2, keep, c2b)
            fgg = fgv[:, t0:t0 + nt]
            fk = t23(shape=(128, nt))
            nc.vector.tensor_mul(fk, fgg, keep2)
            pv = t23(shape=(128, nt))
            nc.vector.tensor_sub(pv, fgg, fk)
            t3 = t23(shape=(128, nt))
            nc.vector.tensor_mul(t3, pv, dd[:, t0:t0 + nt])
            nc.vector.tensor_add(pps[:, t0:t0 + nt], t3, b1[:, t0:t0 + nt])

        rowsum = sb.tile([128, 1], F32)
        nc.vector.reduce_sum(rowsum, pps, axis=AXX)
        nc.sync.dma_start(out=out, in_=rowsum)

    nc.compile()
    return nc


def _prep(features, average_features, outputs, labels_onehot, weights):
    import ml_dtypes
    bf16 = ml_dtypes.bfloat16
    feats = np.asarray(features, np.float32).reshape(ROWS, D)
    z = np.asarray(outputs, np.float32).reshape(ROWS, C)
    lab = np.asarray(labels_onehot, np.float32)
    w = np.asarray(weights, np.float32)
    avg = np.asarray(average_features, np.float32)

    l_img = np.argmax(lab, axis=1)
    lp = np.repeat(l_img, P)                                    # [23040]
    an = avg / np.maximum(np.linalg.norm(avg, axis=1, keepdims=True), 1e-8)

    npad = NCORES * R
    zp = np.zeros((npad, C), np.float32)
    zp[:ROWS] = z
    meta = np.zeros((npad, 8), np.float32)
    meta[:ROWS, 0:4] = np.eye(C, dtype=np.float32)[lp]
    meta[:ROWS, 4] = w[lp]
    meta[:ROWS, 5] = (lp > 0).astype(np.float32)
    meta[:ROWS, 6] = w[0]

    avgtn = np.ascontiguousarray(
        an.T.reshape(K, 128, C).transpose(1, 0, 2).reshape(128, K * C)
    ).astype(bf16)
    eye5 = np.eye(5, dtype=np.float32)

    # Packed feature layout: per core, per group (off, w):
    # featg[p, goff + k*w + r] = feats[core*R + off + r, k*128 + p]
    featsP = np.zeros((npad, D), bf16)
    featsP[:ROWS] = feats.astype(bf16)
    u = featsP.reshape(NCORES, R, K, 128)                       # [core, r, k, p]
    parts = []
    for off, w in GROUPS:
        blk = u[:, off:off + w]                                 # [core, w, k, p]
        parts.append(blk.transpose(0, 3, 2, 1).reshape(NCORES, 128, K * w))
    featg_all = np.ascontiguousarray(np.concatenate(parts, axis=2))

    in_maps = []
    for ci in range(NCORES):
        lo, hi = ci * R, (ci + 1) * R
        zrow = np.ascontiguousarray(
            zp[lo:hi].reshape(RT, 128, C).transpose(1, 0, 2).reshape(128, RT * C))
        metar = np.ascontiguousarray(
            meta[lo:hi].reshape(RT, 128, 8).transpose(1, 0, 2).reshape(128, RT * 8))
        in_maps.append({"featg": featg_all[ci], "avgtn": avgtn,
                        "zrow": zrow, "meta": metar, "eye5": eye5})
    return in_maps


def _install_ntff_hook():
    """Make run_bass_kernel_spmd(trace=True) work in the axon agent container:
    inject the missing antenv.axon_hooks module backed by the boot shim's
    ctypes NTFF driver, and keep artifact upload local."""
    import sys, types
    if "antenv.axon_hooks" in sys.modules:
        return
    import trn_agent_boot.trn_boot as tb
    hook = tb._ntff_profile_via_ctypes("/opt/axon/libaxon_pjrt.so")
    mod = types.ModuleType("antenv.axon_hooks")
    mod.get_axon_ntff_profile_hook = lambda: hook
    mod.set_axon_ntff_profile_hook = lambda h: None
    sys.modules["antenv.axon_hooks"] = mod
    import concourse.bass_utils as bu
    bu.upload_artifacts = lambda tmpdir: tmpdir


def kernel(features, average_features, outputs, labels_onehot, weights,
           _trace=False, _trace_kwargs=None):
    if "nc" not in _CACHE:
        _CACHE["nc"] = _build()
    nc = _CACHE["nc"]
    in_maps = _prep(features, average_features, outputs, labels_onehot, weights)
    kwargs = {}
    if _trace:
        _install_ntff_hook()
        kwargs = dict(trace=True, **(_trace_kwargs or {}))
    res = run_bass_kernel_spmd(nc, in_maps, core_ids=list(range(NCORES)), **kwargs)
    total = np.float64(0.0)
    for r in res.results:
        total += np.float64(r["out"].sum())
    _CACHE["last_results"] = res
    return np.float32(total / ROWS)


# revision 8
# speedup vs baseline: 192546.0331x; 1.0879x over previous
"""Trainium2 Bass kernel for nn_CosineLoss (cosine-similarity pseudo-label CE loss).

Data-parallel over the flattened (B*P) patch dimension across 8 NeuronCores.

Per core the device computes, for each patch x (row of features):
  q_c  = dot(x, a_c / ||a_c||)   for the 4 prototypes   (PE, bf16 matmuls)
  t2   = 0.36 * ||x||^2          (bf16 squares on ACT/DVE/GPSIMD, reduced on
                                  PE against a 0.36-scaled ones vector)
  keep = (q_0 > q_l) & (q_0*|q_0| > t2)                   [== sim_back>sim_sea
                                                           & sim_back>0.6]
  pseudo = is_foreground & ~keep
  s    = softmax(z); lse2 = log(sum(exp(s)))            (double-softmax CE)
  pp   = pseudo ? w_l*(lse2-s_l) : w_0*(lse2-s_0)       (masked for padding)
summed on-device to a single scalar per core; the host adds the 8 scalars and
divides by B*P.

Features are converted to bf16 on the host (the loss only consumes them
through three comparisons with huge margins, so bf16 is far inside the
accuracy budget) and pre-packed so the contraction dim D lands on SBUF
partitions; this halves HBM traffic and doubles PE streaming rate vs fp32.

Both matmul passes per row-group share one [5, w] PSUM tile: the q pass uses
stationary [a0..a3 | 0] with start=True (clearing all 5 rows), the norm pass
uses [0 0 0 0 | 0.36*ones] accumulating row 4 only.
"""

import numpy as np
from contextlib import ExitStack

import concourse.bass as bass
import concourse.bacc as bacc
import concourse.tile as tile
from concourse import mybir
from concourse.bass_utils import run_bass_kernel_spmd

# Problem constants (hardcoded; kernel.py must be self-contained).
B, P, D, C = 512, 45, 2048, 4
THRESH2 = 0.36  # THRESH**2, THRESH = 0.6
NCORES = 8
ROWS = B * P                 # 23040 patches
RT = 23                      # row tiles of 128 per core
R = RT * 128                 # 2944 padded rows per core
K = D // 128                 # 16 contraction chunks
GROUPS = [(0, 512), (512, 512), (1024, 512), (1536, 512), (2048, 512), (2560, 384)]

F32 = mybir.dt.float32
BF16 = mybir.dt.bfloat16
AF = mybir.ActivationFunctionType
ALU = mybir.AluOpType
AXX = mybir.AxisListType.X

_CACHE = {}


def _build():
    nc = bacc.Bacc("TRN2", target_bir_lowering=False, debug=False)
    gsz = K * R
    featg = nc.dram_tensor("featg", [128, gsz], BF16, kind="ExternalInput").ap()
    avgtn = nc.dram_tensor("avgtn", [128, K * 5], BF16, kind="ExternalInput").ap()
    zrow = nc.dram_tensor("zrow", [128, RT * C], F32, kind="ExternalInput").ap()
    meta = nc.dram_tensor("meta", [128, RT * 8], F32, kind="ExternalInput").ap()
    eye5 = nc.dram_tensor("eye5", [5, 5], F32, kind="ExternalInput").ap()
    out = nc.dram_tensor("out", [1, 1], F32, kind="ExternalOutput").ap()

    with tile.TileContext(nc) as tc, ExitStack() as ctx:
        consts = ctx.enter_context(tc.tile_pool(name="consts", bufs=1))
        gpool = ctx.enter_context(tc.tile_pool(name="gpool", bufs=3))
        sqpool = ctx.enter_context(tc.tile_pool(name="sqpool", bufs=3))
        sb = ctx.enter_context(tc.tile_pool(name="sb", bufs=1))
        qps = ctx.enter_context(tc.tile_pool(name="qps", bufs=2, space="PSUM"))
        tps = ctx.enter_context(tc.tile_pool(name="tps", bufs=2, space="PSUM"))

        _tcnt = [0]

        def t23(pool=sb, shape=(128, RT), dt=F32):
            _tcnt[0] += 1
            nm = f"tmp_{_tcnt[0]}"
            return pool.tile(list(shape), dt, name=nm, tag=nm)

        # ---- feature stream DMAs first (SP HWDGE ring) ----
        gtiles = []
        goff = 0
        for gi, (off, w) in enumerate(GROUPS):
            g = gpool.tile([128, K, w], BF16, name=f"g{gi}", tag="g")
            gtiles.append(g)
            nsub = 8 if gi == 0 else 4
            sub = K // nsub
            for s in range(nsub):
                nc.sync.dma_start(
                    out=g[:, s * sub:(s + 1) * sub, :].rearrange("p k r -> p (k r)"),
                    in_=featg[:, goff + s * sub * w: goff + (s + 1) * sub * w])
            goff += K * w

        # ---- small constants on the ACT HWDGE ring (parallel trigger issue) ----
        # avg5: [a0..a3 | 0] per chunk; z15: [0,0,0,0 | 0.36] (threshold folded in)
        avg5_sb = consts.tile([128, K, 5], BF16)
        nc.scalar.dma_start(out=avg5_sb, in_=avgtn.rearrange("p (k c) -> p k c", c=5))
        eye5_sb = consts.tile([5, 5], F32)
        nc.scalar.dma_start(out=eye5_sb, in_=eye5)
        z15 = consts.tile([128, 5], BF16)
        nc.vector.memset(z15[:, 0:4], 0.0)
        nc.vector.memset(z15[:, 4:5], THRESH2)
        ones32 = consts.tile([128, 1], F32)
        nc.vector.memset(ones32, 1.0)
        zsb = sb.tile([128, RT, C], F32)
        nc.scalar.dma_start(out=zsb, in_=zrow.rearrange("p (t c) -> p t c", c=C))
        msb = sb.tile([128, RT, 8], F32)
        nc.scalar.dma_start(out=msb, in_=meta.rearrange("p (t c) -> p t c", c=8))

        oh = msb[:, :, 0:4]
        wl = msb[:, :, 4]
        fgv = msb[:, :, 5]
        w0v = msb[:, :, 6]

        # ---- z-only epilogue half, hoisted to the front (overlaps group DMAs,
        # and pulls the ACT exp/ln table loads off the tail) ----
        e = sb.tile([128, RT, C], F32)
        nc.scalar.activation(e, zsb, AF.Exp)
        zsum = t23()
        nc.vector.reduce_sum(zsum, e, axis=AXX)
        rz = t23()
        nc.vector.reciprocal(rz, zsum)
        s = sb.tile([128, RT, C], F32)
        nc.vector.tensor_mul(s, e, rz.unsqueeze(2).broadcast_to([128, RT, C]))
        es = sb.tile([128, RT, C], F32)
        nc.scalar.activation(es, s, AF.Exp)
        essum = t23()
        nc.vector.reduce_sum(essum, es, axis=AXX)
        lse2 = t23()
        nc.scalar.activation(lse2, essum, AF.Ln)
        soh = sb.tile([128, RT, C], F32)
        nc.vector.tensor_mul(soh, s, oh)
        sl = t23()
        nc.vector.reduce_sum(sl, soh, axis=AXX)
        base = t23()
        nc.vector.tensor_sub(base, lse2, s[:, :, 0])
        alt = t23()
        nc.vector.tensor_sub(alt, lse2, sl)
        b1 = t23()
        nc.vector.tensor_mul(b1, w0v, base)
        a1 = t23()
        nc.vector.tensor_mul(a1, wl, alt)
        dd = t23()
        nc.vector.tensor_sub(dd, a1, b1)
        # pp = bd - fgd*keep with fgd = fg*dd, bd = b1 + fgd (hoisted off the tail)
        fgd = t23()
        nc.vector.tensor_mul(fgd, fgv, dd)
        bd = t23()
        nc.vector.tensor_add(bd, b1, fgd)

        # ---- main feature stream: q matmuls -> squares -> n2 matmuls ->
        # stage -> transpose -> per-group epilogue ----
        pps = sb.tile([128, RT], F32)
        sq_engines = [nc.scalar, nc.scalar, nc.vector, nc.gpsimd]
        for gi, (off, w) in enumerate(GROUPS):
            g = gtiles[gi]
            pq = qps.tile([5, w], F32, name=f"pq{gi}", tag="pq")
            for k in range(K):
                nc.tensor.matmul(pq, avg5_sb[:, k, :], g[:, k, :],
                                 start=(k == 0), stop=False)
            sq = sqpool.tile([128, K, w], BF16, name=f"sq{gi}", tag="sq")
            for j in range(4):
                src = g[:, j * 4:(j + 1) * 4, :]
                dst = sq[:, j * 4:(j + 1) * 4, :]
                eng = sq_engines[j]
                if eng is nc.scalar:
                    eng.activation(dst, src, AF.Square)
                else:
                    eng.tensor_mul(dst, src, src)
            for k in range(K):
                nc.tensor.matmul(pq, z15, sq[:, k, :],
                                 start=False, stop=(k == K - 1))
            stq = t23(shape=(5, w))
            nc.vector.tensor_copy(stq, pq)
            # per-group small transposes: [5, w] -> w/128 tiles of [128, 5]
            nt = w // 128
            t0 = off // 128
            ptq = tps.tile([128, nt * 5], F32, name=f"ptq{gi}", tag="ptq")
            for j in range(nt):
                nc.tensor.transpose(ptq[:, j * 5:(j + 1) * 5],
                                    stq[:, j * 128:(j + 1) * 128], eye5_sb)
            qn = t23(shape=(128, nt, 5))
            nc.vector.tensor_copy(qn, ptq.rearrange("p (t c) -> p t c", c=5))

            # ---- q-dependent epilogue for this group's row tiles ----
            q0 = qn[:, :, 0]
            qoh = t23(shape=(128, nt, 4))
            nc.vector.tensor_mul(qoh, qn[:, :, 0:4], oh[:, t0:t0 + nt, :])
            ql = t23(shape=(128, nt))
            nc.vector.reduce_sum(ql, qoh, axis=AXX)
            c1 = t23(shape=(128, nt))
            nc.vector.tensor_tensor(c1, q0, ql, op=ALU.is_gt)
            q0sq = t23(shape=(128, nt))
            nc.vector.tensor_mul(q0sq, q0, q0)
            c2a = t23(shape=(128, nt))
            nc.vector.tensor_scalar(c2a, q0, 0.0, None, op0=ALU.is_gt)
            c2b = t23(shape=(128, nt))
            nc.vector.tensor_tensor(c2b, q0sq, qn[:, :, 4], op=ALU.is_gt)
            m1 = t23(shape=(128, nt))
            nc.vector.tensor_mul(m1, c1, c2a)
            k2 = t23(shape=(128, nt))
            nc.vector.tensor_mul(k2, m1, c2b)
            f = t23(shape=(128, nt))
            nc.vector.tensor_mul(f, fgd[:, t0:t0 + nt], k2)
            nc.vector.tensor_sub(pps[:, t0:t0 + nt], bd[:, t0:t0 + nt], f)

        # ---- final reduction to a single scalar; tiny single-run output DMA ----
        rowsum = sb.tile([128, 1], F32)
        nc.vector.reduce_sum(rowsum, pps, axis=AXX)
        po = tps.tile([1, 1], F32, name="po", tag="po")
        nc.tensor.matmul(po, ones32, rowsum, start=True, stop=True)
        sc = sb.tile([1, 1], F32)
        nc.vector.tensor_copy(sc, po)
        nc.sync.dma_start(out=out, in_=sc)

    nc.compile()
    return nc


def _prep(features, average_features, outputs, labels_onehot, weights):
    import ml_dtypes
    bf16 = ml_dtypes.bfloat16
    feats = np.asarray(features, np.float32).reshape(ROWS, D)
    z = np.asarray(outputs, np.float32).reshape(ROWS, C)
    lab = np.asarray(labels_onehot, np.float32)
    w = np.asarray(weights, np.float32)
    avg = np.asarray(average_features, np.float32)

    l_img = np.argmax(lab, axis=1)
    lp = np.repeat(l_img, P)                                    # [23040]
    an = avg / np.maximum(np.linalg.norm(avg, axis=1, keepdims=True), 1e-8)

    npad = NCORES * R
    zp = np.zeros((npad, C), np.float32)
    zp[:ROWS] = z
    meta = np.zeros((npad, 8), np.float32)
    meta[:ROWS, 0:4] = np.eye(C, dtype=np.float32)[lp]
    meta[:ROWS, 4] = w[lp]
    meta[:ROWS, 5] = (lp > 0).astype(np.float32)
    meta[:ROWS, 6] = w[0]

    # avg5: per chunk 5 columns = [a0..a3 | 0]
    an5 = np.zeros((D, 5), np.float32)
    an5[:, 0:4] = an.T
    avgtn = np.ascontiguousarray(
        an5.reshape(K, 128, 5).transpose(1, 0, 2).reshape(128, K * 5)
    ).astype(bf16)
    eye5 = np.eye(5, dtype=np.float32)

    # Packed feature layout: per core, per group (off, w):
    # featg[p, goff + k*w + r] = feats[core*R + off + r, k*128 + p]
    featsP = np.zeros((npad, D), bf16)
    featsP[:ROWS] = feats.astype(bf16)
    u = featsP.reshape(NCORES, R, K, 128)                       # [core, r, k, p]
    parts = []
    for off, w in GROUPS:
        blk = u[:, off:off + w]                                 # [core, w, k, p]
        parts.append(blk.transpose(0, 3, 2, 1).reshape(NCORES, 128, K * w))
    featg_all = np.ascontiguousarray(np.concatenate(parts, axis=2))

    in_maps = []
    for ci in range(NCORES):
        lo, hi = ci * R, (ci + 1) * R
        zrow = np.ascontiguousarray(
            zp[lo:hi].reshape(RT, 128, C).transpose(1, 0, 2).reshape(128, RT * C))
        metar = np.ascontiguousarray(
            meta[lo:hi].reshape(RT, 128, 8).transpose(1, 0, 2).reshape(128, RT * 8))
        in_maps.append({"featg": featg_all[ci], "avgtn": avgtn,
                        "zrow": zrow, "meta": metar, "eye5": eye5})
    return in_maps


def _install_ntff_hook():
    """Make run_bass_kernel_spmd(trace=True) work in the axon agent container:
    inject the missing antenv.axon_hooks module backed by the boot shim's
    ctypes NTFF driver, and keep artifact upload local."""
    import sys, types
    if "antenv.axon_hooks" in sys.modules:
        return
    import trn_agent_boot.trn_boot as tb
    hook = tb._ntff_profile_via_ctypes("/opt/axon/libaxon_pjrt.so")
    mod = types.ModuleType("antenv.axon_hooks")
    mod.get_axon_ntff_profile_hook = lambda: hook
    mod.set_axon_ntff_profile_hook = lambda h: None
    sys.modules["antenv.axon_hooks"] = mod
    import concourse.bass_utils as bu
    bu.upload_artifacts = lambda tmpdir: tmpdir


def kernel(features, average_features, outputs, labels_onehot, weights,
           _trace=False, _trace_kwargs=None):
    if "nc" not in _CACHE:
        _CACHE["nc"] = _build()
    nc = _CACHE["nc"]
    in_maps = _prep(features, average_features, outputs, labels_onehot, weights)
    kwargs = {}
    if _trace:
        _install_ntff_hook()
        kwargs = dict(trace=True, **(_trace_kwargs or {}))
    res = run_bass_kernel_spmd(nc, in_maps, core_ids=list(range(NCORES)), **kwargs)
    total = np.float64(0.0)
    for r in res.results:
        total += np.float64(r["out"][0, 0])
    _CACHE["last_results"] = res
    return np.float32(total / ROWS)
